# revision 33
# baseline (speedup 1.0000x reference)
"""Trainium2 Bass kernel for nn_ExplicitCircuit (12-qubit batched statevector sim).

Math: the circuit's prefix (H on all qubits + diagonal data-dependent
PhaseShift/IsingZZ) collapses to state[b,s] = (1/64) * exp(i*phi(b,s)) with
phi a rank-30 factorized matmul computed directly in the on-chip layout.
The three variational layers are 6 group unitaries (Kron products of
per-qubit RZ*RY*RX, 64x64 complex, host-built from the 108 weights) applied on
the TensorEngine; each application uses the state as the matmul *stationary*
operand which simultaneously transposes the layout, so gates alternate between
the two qubit groups with zero explicit transposes. Ring-CZ entanglers fold
into the gate matrices except two group-crossing edges applied as a +-1 mask.
Measurement of <Z_0> is |amp|^2 reduction with a +-1 partition matmul.

Data parallel: batch 256 -> 8 cores x 32. Weights/constants replicated.

Internal conventions:
  state index s: qubit q <-> bit q of s.  a = s & 63, h = s >> 6.
  per-core batch b = c*16 + j*2 + beta  (c,beta in {0,1}, j in [0,8))
  Layout X: partition p = c*64 + a,    free f = j*128 + beta*64 + h
  Layout Y: partition p = beta*64 + h, free f = j*128 + c*64 + a
  Gate application maps X->Y->X->...

v9 schedule vs v1 baseline (33.6us -> 22.8us cost-model):
  - phi pack compressed to K=30 f32r (two small DMAs vs one 1.8us one;
    f32r matmul 1 cyc/row vs fp32's 4) -> trig starts ~2.7us earlier.
  - the final B-group gate (and its CZ diag) commutes with the Z_0
    measurement and preserves per-(b,a) h-norms -> dropped entirely
    (16 matmuls + one full boundary + a shorter tail).
  - measurement done in Y layout: ACT squares gate-4's psum banks, DVE
    reduces (z, a-even) / (z, a-odd) separately via strided 5-D views
    and subtracts; the final per-beta partition sums run as two GPSIMD
    C-axis reduces straight into the output staging row (no PE
    round-trip).
  - state between gates stored interleaved chunk-major ([re|im] per
    128-col chunk) so boundary traffic is flat full-bank moves.
  - hw hazard found on the way: ACT and DVE must not read the same PSUM
    bank concurrently (faults the exec unit) -> boundary banks are
    partitioned between engines (DVE: 0,2 / ACT: 1,3 on clean
    boundaries; masked boundaries all-DVE); GPSIMD cannot touch PSUM.
"""
import numpy as np

NQ = 12
NL = 3
BL = 32
NCORES = 8

PAIRS = [(i, j) for i in range(NQ) for j in range(i + 1, NQ)]
PAIR_IDX = {p: k for k, p in enumerate(PAIRS)}

_a = np.arange(64)
_h = np.arange(64)
BIT_A = ((_a[None, :] >> np.arange(6)[:, None]) & 1).astype(np.float64)
BIT_H = ((_h[None, :] >> np.arange(6)[:, None]) & 1).astype(np.float64)
CHI_A = 1.0 - 2.0 * BIT_A
CHI_H = 1.0 - 2.0 * BIT_H

TWO_PI = 2.0 * np.pi
INV_TWO_PI = np.float32(1.0 / TWO_PI)
MAGIC = np.float32(1.5 * 2.0 ** 23)  # keeps result in [2^23, 2^24) where ulp=1
FL2PI = float(np.float32(TWO_PI))

# AA pair coef matrix [15 pairs -> (i, ip)], precomputed a-side patterns
AA_PAIRS = [(i, j) for (i, j) in PAIRS if j <= 5]
BB_PAIRS = [(i, j) for (i, j) in PAIRS if i >= 6]
AB_PAIRS = [(i, j) for (i, j) in PAIRS if i < 6 <= j]


def build_lr30(x_core):
    """lhsT [30, 128], rhs [30, 1024] with phi = lhsT.T @ rhs in layout X.

    Rows 0:16  (K=(j',beta')): a-only terms, host-contracted into
               phiA_c[a, (j,beta)]; rhs is the batch indicator pattern.
    Rows 16:30 (7 per c): [chiA_0..5, 1] x [hcoef rows, phiB row].
    """
    x = np.asarray(x_core, np.float64)  # [32, 78]
    lhsT = np.zeros((30, 128))
    rhs = np.zeros((30, 1024))

    # a-side per-batch coefficient stack: [27ish terms] -> phiA [b, a]
    # phiA[b, a] = sum_i<6 x[b,i] bitA_i(a) + sum_AA -0.5 x chi chi
    phiA = x[:, 0:6] @ BIT_A  # [32, 64]
    for (i, ip) in AA_PAIRS:
        phiA += np.outer(-0.5 * x[:, 12 + PAIR_IDX[(i, ip)]], CHI_A[i] * CHI_A[ip])
    for c in range(2):
        for k in range(16):
            b = c * 16 + k
            lhsT[k, c * 64:(c + 1) * 64] = phiA[b]
    for k in range(16):
        jj, beta = k // 2, k % 2
        rhs[k, jj * 128 + beta * 64: jj * 128 + beta * 64 + 64] = 1.0

    # cross terms: hcoef[b, t, h] = sum_{j>=6} -0.5 x[b, pair(t,j)] chiH_{j-6}
    hcoef = np.zeros((BL, 6, 64))
    for (i, j) in AB_PAIRS:
        hcoef[:, i, :] += np.outer(-0.5 * x[:, 12 + PAIR_IDX[(i, j)]], CHI_H[j - 6])
    phiB = x[:, 6:12] @ BIT_H
    for (i, j) in BB_PAIRS:
        phiB += np.outer(-0.5 * x[:, 12 + PAIR_IDX[(i, j)]], CHI_H[i - 6] * CHI_H[j - 6])

    for c in range(2):
        base = 16 + 7 * c
        for t in range(6):
            lhsT[base + t, c * 64:(c + 1) * 64] = CHI_A[t]
        lhsT[base + 6, c * 64:(c + 1) * 64] = 1.0
        for jj in range(8):
            for beta in range(2):
                b = c * 16 + jj * 2 + beta
                f0 = jj * 128 + beta * 64
                rhs[base + 0:base + 6, f0:f0 + 64] = hcoef[b]
                rhs[base + 6, f0:f0 + 64] = phiB[b]
    out = np.zeros((30, 1152), np.float32)
    out[:, 0:128] = lhsT
    out[:, 128:1152] = rhs
    return out


def _rx(t):
    c, s = np.cos(t / 2), np.sin(t / 2)
    return np.array([[c, -1j * s], [-1j * s, c]])


def _ry(t):
    c, s = np.cos(t / 2), np.sin(t / 2)
    return np.array([[c, -s], [s, c]])


def _rz(t):
    return np.diag([np.exp(-0.5j * t), np.exp(0.5j * t)])


def _kron_chain(mats):
    out = np.array([[1.0 + 0j]])
    for m in mats:
        out = np.kron(m, out)
    return out


def _cz_diag(bits):
    d = np.ones(64)
    for i in range(5):
        d *= 1.0 - 2.0 * (bits[i] * bits[i + 1])
    return d


def effective_gates(weights):
    w = np.asarray(weights, np.float64)
    UAs, UBs = [], []
    p = 0
    for _l in range(NL):
        mats = []
        for _q in range(NQ):
            mats.append(_rz(w[p + 2]) @ _ry(w[p + 1]) @ _rx(w[p]))
            p += 3
        UAs.append(_kron_chain(mats[0:6]))
        UBs.append(_kron_chain(mats[6:12]))
    dA = _cz_diag(BIT_A)
    # Gate 5 (UBs[2] and ring-2's B-chain CZ diag) acts only on the h
    # qubits; it commutes with Z_0 (an a-side observable) and preserves
    # per-(b,a) h-norms, so it drops out of the expectation entirely.
    dB = _cz_diag(BIT_H)
    return [UAs[0] / 64.0, UBs[0],
            UAs[1] * dA[None, :], UBs[1] * dB[None, :],
            UAs[2] * dA[None, :]]


def pack_gates_packed(weights):
    """[128, 2560]: per gate g: M1 = [Wr | Wi], M2 = [-Wi | Wr], each 128x256."""
    G = effective_gates(weights)
    out = np.zeros((128, 5 * 512), np.float32)
    eye2 = np.eye(2)
    for g, Gm in enumerate(G):
        W = np.kron(eye2, Gm.T)
        base = g * 512
        out[:, base + 0:base + 128] = W.real
        out[:, base + 128:base + 256] = W.imag
        out[:, base + 256:base + 384] = -W.imag
        out[:, base + 384:base + 512] = W.real
    return out


def cz_mask_X():
    """Interleaved chunk-major mask [128, 2048]: per chunk jj the 256 cols
    are [re 128 | im 128], both carrying the same +-1 cross-CZ pattern."""
    dx = np.ones((64, 64))
    dx *= 1.0 - 2.0 * np.outer(BIT_A[5], BIT_H[0])
    dx *= 1.0 - 2.0 * np.outer(BIT_A[0], BIT_H[5])
    m = np.zeros((128, 2048), np.float32)
    for c in range(2):
        for jj in range(8):
            for z in range(2):
                for beta in range(2):
                    f0 = jj * 256 + z * 128 + beta * 64
                    m[c * 64:(c + 1) * 64, f0:f0 + 64] = dx
    return m


def sign_vec():
    """[128, 4]: beta-indicator columns (+1) then negated copies (-1)."""
    sv = np.zeros((128, 4), np.float32)
    sv[0:64, 0] = 1.0
    sv[64:128, 1] = 1.0
    sv[:, 2:4] = -sv[:, 0:2]
    return sv


# ----------------------------- device program -----------------------------

_CACHE = {}

N_WARM = 10


def _build_nc():
    import concourse.bass as bass
    import concourse.mybir as mybir

    fp32 = mybir.dt.float32
    f32r = mybir.dt.float32r
    Alu = mybir.AluOpType
    Act = mybir.ActivationFunctionType
    nc = bass.Bass()

    lr_d = nc.dram_tensor("lr", [30, 1152], f32r, kind="ExternalInput")
    gates_d = nc.dram_tensor("gates", [128, 2560], f32r, kind="ExternalInput")
    aux_d = nc.dram_tensor("aux", [128, 2052], fp32, kind="ExternalInput")
    y_d = nc.dram_tensor("y", [2, 16], fp32, kind="ExternalOutput")

    # --- tick ledger (computed below as streams are declared) ---
    # s_pe: phi 1,2; gate g chunk jj second matmul = 2+16g+2(jj+1); final = 99
    def pe_chunk(g, jj):
        return 2 + 16 * g + 2 * (jj + 1)

    from contextlib import ExitStack
    with ExitStack() as stack:
        ent = stack.enter_context
        lr_sb = ent(nc.sbuf_tensor("lr_sb", [30, 1152], f32r))
        gsb = ent(nc.sbuf_tensor("gsb", [128, 2560], f32r))
        aux_sb = ent(nc.sbuf_tensor("aux_sb", [128, 2052], fp32))
        SRa = ent(nc.sbuf_tensor("sra", [128, 1024], f32r))
        SIa = ent(nc.sbuf_tensor("sia", [128, 1024], f32r))
        SX = ent(nc.sbuf_tensor("sx", [128, 2048], f32r))
        SY = ent(nc.sbuf_tensor("sy", [128, 2048], f32r))
        t1 = ent(nc.sbuf_tensor("t1", [128, 1024], fp32))
        t2 = ent(nc.sbuf_tensor("t2", [128, 1024], fp32))
        rr = ent(nc.sbuf_tensor("rr", [128, 16], fp32))
        rr2 = ent(nc.sbuf_tensor("rr2", [128, 32], fp32))
        ysb = ent(nc.sbuf_tensor("ysb", [1, 32], fp32))
        wz = ent(nc.sbuf_tensor("wz", [128, 64], fp32))
        hp = ent(nc.sbuf_tensor("hp", [128, 1], fp32))
        ps0 = ent(nc.psum_tensor("ps0", [128, 2048], fp32))
        ps1 = ent(nc.psum_tensor("ps1", [128, 2048], fp32))
        s_gp = ent(nc.semaphore("s_gp"))
        d1 = ent(nc.semaphore("d1"))
        d2 = ent(nc.semaphore("d2"))
        s_pe = ent(nc.semaphore("s_pe"))
        s_dve = ent(nc.semaphore("s_dve"))
        s_act = ent(nc.semaphore("s_act"))
        block = ent(nc.Block())
        lr_ap = lr_sb.ap()
        g_ap = gsb.ap()
        aux_ap = aux_sb.ap()
        m_sb = aux_ap[:, 0:2048]
        svp = aux_ap[:, 2048:2050]
        svn = aux_ap[:, 2050:2052]
        pairs = [ps0.ap(), ps1.ap()]
        phi = pairs[1][:, 0:1024]
        sx, sy = SX.ap(), SY.ap()
        t1a, t2a = t1.ap(), t2.ap()

        def q(ap_, k):  # 256-col quarter view
            return ap_[:, k * 256:(k + 1) * 256]

        def r_piece(ps, lo, hi):
            v = ps.rearrange("p (c x) -> p c x", x=256)
            return v[:, lo:hi, 0:128]

        def i_piece(ps, lo, hi):
            v = ps.rearrange("p (c x) -> p c x", x=256)
            return v[:, lo:hi, 128:256]

        def st_piece(sap, lo, hi):
            v = sap.rearrange("p (c x) -> p c x", x=128)
            return v[:, lo:hi, :]

        def msk_piece(lo, hi):
            v = m_sb.rearrange("p (c x) -> p c x", x=128)
            return v[:, lo:hi, :]

        # ------------------------ tick ledger -------------------------
        # HW hazard: ACT and DVE must never read the same PSUM bank
        # concurrently -> boundary traffic is partitioned by 512-col bank
        # (quarter), with the state kept interleaved chunk-major
        # ([re|im] per 128-col chunk) so each boundary piece is one flat
        # full-bank copy.  Gate g>=1 stationary chunks slice SX/SY.
        # s_pe:  phi=1,2; gate g (0..4) chunk jj -> pe_chunk(g,jj)=2+16g+2(jj+1)
        #        (max 82); final mm=83.
        # s_dve: trig q0:1-3 q1:4-6 q2:7-10 q3:11-14
        #        b0 q0=15 q2=16 | b1 mults q0..3 = 17-20
        #        b2 q0=21 q2=22 | b3 mults q0..3 = 23-26
        #        tail red_e/o q0..3 = 27-34, sub=35
        # s_act: trig abs0=1,i0=2,r0=3,abs1=4,i1=5,r1=6,i2=7,r2=8,i3=9,r3=10
        #        b0 q1=11 q3=12 | b2 q1=13 q3=14
        #        tail sq q0=15 q1=16 q2=17 q3c6=18 q3c7=19
        # s_gp:  wz=1, hp=2; beta C-reduces into ysb = 3,4
        # tail sq slabs: q0->t1[0:512] q1->t1[512:] q2->t2[512:] q3->t2[0:512]

        bnd = [(0, sx), (1, sy), (2, sx), (3, sy)]

        @block.gpsimd
        def _(gpsimd):
            nc.gpsimd.memset(wz.ap(), 0.0).then_inc(s_gp, 1)
            nc.gpsimd.memset(hp.ap(), float(np.pi / 2)).then_inc(s_gp, 1)
            gpsimd.wait_ge(s_dve, 35)
            for beta in range(2):
                nc.gpsimd.tensor_reduce(
                    ysb.ap()[0:1, 16 * beta:16 * beta + 16],
                    rr.ap()[64 * beta:64 * (beta + 1), :],
                    mybir.AxisListType.C, Alu.add).then_inc(s_gp, 1)

        @block.sync
        def _(sync):
            sync.dma_start(lr_ap[:, 0:640], lr_d[:, 0:640]).then_inc(d1, 16)
            sync.dma_start(lr_ap[:, 640:1152], lr_d[:, 640:1152]).then_inc(d1, 16)
            sync.dma_start(g_ap[:, 0:1024], gates_d[:, 0:1024]).then_inc(d2, 16)
            sync.dma_start(aux_ap, aux_d[:]).then_inc(d2, 16)
            sync.dma_start(g_ap[:, 1024:2560], gates_d[:, 1024:2560]).then_inc(d2, 16)
            sync.wait_ge(s_gp, 4)
            sync.dma_start(y_d[:], ysb.ap()).then_inc(d1, 16)

        @block.tensor
        def _(tensor):
            # warm-up: establish pe_busy_start early so phi/gates run at
            # full clock; results discarded.
            tensor.wait_ge(s_gp, 1)
            for _ in range(N_WARM):
                nc.tensor.matmul(pairs[0][0:64, 0:64], wz.ap(), wz.ap(),
                                 start=True, stop=True)
            for half in range(2):
                tensor.wait_ge(d1, 16 * (half + 1))
                nc.tensor.matmul(
                    phi[:, half * 512:(half + 1) * 512],
                    lr_ap[0:30, 0:128],
                    lr_ap[0:30, 128 + half * 512:128 + (half + 1) * 512],
                    start=True, stop=True,
                ).then_inc(s_pe, 1)
            g_waits = {
                (0, 0): [(d2, 16), (s_act, 3)],
                (0, 2): [(s_act, 6)],
                (0, 4): [(s_act, 8)],
                (0, 6): [(s_act, 10)],
                (1, 0): [(s_dve, 15)],
                (1, 2): [(s_act, 11)],
                (1, 4): [(s_dve, 16)],
                (1, 6): [(s_act, 12)],
                (2, 0): [(d2, 48), (s_dve, 17)],
                (2, 2): [(s_dve, 18)],
                (2, 4): [(s_dve, 19)],
                (2, 6): [(s_dve, 20)],
                (3, 0): [(s_dve, 21)],
                (3, 2): [(s_act, 13)],
                (3, 4): [(s_dve, 22)],
                (3, 6): [(s_act, 14)],
                (4, 0): [(s_dve, 23)],
                (4, 2): [(s_dve, 24)],
                (4, 4): [(s_dve, 25)],
                (4, 6): [(s_dve, 26)],
            }
            for g in range(5):
                m1 = g_ap[:, g * 512:g * 512 + 256]
                m2 = g_ap[:, g * 512 + 256:g * 512 + 512]
                ps = pairs[g % 2]
                st_in = None if g == 0 else (sx if g % 2 == 1 else sy)
                for jj in range(8):
                    for sem, tick in g_waits.get((g, jj), []):
                        tensor.wait_ge(sem, tick)
                    if g == 0:
                        cr = SRa.ap()[:, jj * 128:(jj + 1) * 128]
                        ci = SIa.ap()[:, jj * 128:(jj + 1) * 128]
                    else:
                        cr = st_in[:, jj * 256:jj * 256 + 128]
                        ci = st_in[:, jj * 256 + 128:jj * 256 + 256]
                    reg = ps[:, jj * 256:(jj + 1) * 256]
                    nc.tensor.matmul(reg, cr, m1, start=True, stop=False).then_inc(s_pe, 1)
                    nc.tensor.matmul(reg, ci, m2, start=False, stop=True).then_inc(s_pe, 1)


        @block.vector
        def _(vector):
            # trig chains: kf(t1) = phi/2pi + MAGIC; kf -= MAGIC;
            # rt(t2) = phi - 2pi*kf; |rt|(t1) = max(-rt, rt) (q2,q3 only).
            for k in range(4):
                if k == 0:
                    vector.wait_ge(s_pe, 1)
                if k == 2:
                    vector.wait_ge(s_pe, 2)
                nc.vector.tensor_scalar(q(t1a, k), q(phi, k), float(INV_TWO_PI),
                                        float(MAGIC), Alu.mult,
                                        Alu.add).then_inc(s_dve, 1)
                nc.vector.tensor_scalar_sub(q(t1a, k), q(t1a, k),
                                            float(MAGIC)).then_inc(s_dve, 1)
                nc.vector.scalar_tensor_tensor(q(t2a, k), q(t1a, k), -FL2PI,
                                               q(phi, k), Alu.mult,
                                               Alu.add).then_inc(s_dve, 1)
                if k >= 2:
                    nc.vector.scalar_tensor_tensor(
                        q(t1a, k), q(t2a, k), -1.0, q(t2a, k),
                        Alu.mult, Alu.max).then_inc(s_dve, 1)
            # boundary full-bank moves (DVE owns banks 0 and 2; the masked
            # boundaries' bank 1 too -- ACT owns banks 1,3 on clean, 3 via
            # t2+Pool on masked; never the same bank as DVE at once)
            for g, s_out in bnd:
                ps = pairs[g % 2]
                masked = g in (1, 3)
                if g == 1:
                    vector.wait_ge(d2, 32)
                for k in ((0, 1, 2, 3) if masked else (0, 2)):
                    vector.wait_ge(s_pe, pe_chunk(g, 2 * k + 1))
                    dst = s_out[:, 512 * k:512 * (k + 1)]
                    srcp = ps[:, 512 * k:512 * (k + 1)]
                    if masked:
                        nc.vector.tensor_tensor(
                            dst, srcp, m_sb[:, 512 * k:512 * (k + 1)],
                            Alu.mult).then_inc(s_dve, 1)
                    else:
                        nc.vector.tensor_copy(dst, srcp).then_inc(s_dve, 1)
            # tail: gate-4 output in Y layout: psum chunk cols =
            # jj*256 + z*128 + c*64 + a  (z = re/im, c in free, measured
            # bit = a&1).  |amp|^2 summed over (z, a-even/odd) separately,
            # sign applied by one subtract.
            slab = [t1a[:, 0:512], t1a[:, 512:1024], t2a[:, 512:1024], t2a[:, 0:512]]
            for k in range(4):
                v6 = slab[k].rearrange("p (jj z c ae two) -> p jj c z ae two",
                                       z=2, c=2, ae=32, two=2)
                vector.wait_ge(s_act, 15 + k)
                for par in (0, 1):
                    nc.vector.tensor_reduce(
                        rr2.ap()[:, 16 * par + 4 * k:16 * par + 4 * k + 4]
                        .rearrange("p (jj c) -> p jj c", c=2),
                        v6[:, :, :, :, :, par:par + 1].squeeze(-1),
                        mybir.AxisListType.XY, Alu.add).then_inc(s_dve, 1)  # 27-34
            nc.vector.tensor_tensor(rr.ap(), rr2.ap()[:, 0:16],
                                    rr2.ap()[:, 16:32],
                                    Alu.subtract).then_inc(s_dve, 1)  # 35

        @block.scalar
        def _(scalar):
            scalar.wait_ge(s_gp, 2)
            # trig sins; |rt| on ACT for q0,q1
            for k in range(4):
                if k < 2:
                    scalar.wait_ge(s_dve, 3 * k + 3)
                    nc.scalar.activation(q(t1a, k), q(t2a, k),
                                         Act.Abs).then_inc(s_act, 1)
                elif k == 2:
                    scalar.wait_ge(s_dve, 10)
                else:
                    scalar.wait_ge(s_dve, 14)
                nc.scalar.activation(q(SIa.ap(), k), q(t2a, k),
                                     Act.Sin).then_inc(s_act, 1)
                nc.scalar.activation(q(SRa.ap(), k), q(t1a, k),
                                     Act.Sin, bias=hp.ap(),
                                     scale=-1.0).then_inc(s_act, 1)
            # boundary banks 1,3 on clean boundaries only
            for g, s_out in bnd:
                ps = pairs[g % 2]
                if g in (1, 3):
                    continue
                for k in (1, 3):
                    scalar.wait_ge(s_pe, pe_chunk(g, 2 * k + 1))
                    nc.scalar.copy(s_out[:, 512 * k:512 * (k + 1)],
                                   ps[:, 512 * k:512 * (k + 1)]).then_inc(s_act, 1)
            # tail |amp|^2 squares: full banks of gate-4 psum
            slab = [t1a[:, 0:512], t1a[:, 512:1024], t2a[:, 512:1024], t2a[:, 0:512]]
            for k in range(4):
                scalar.wait_ge(s_pe, pe_chunk(4, 2 * k + 1))
                nc.scalar.activation(slab[k], pairs[0][:, 512 * k:512 * (k + 1)],
                                     Act.Square).then_inc(s_act, 1)  # 15-18

    return nc


def _make_in_maps(x, weights):
    gates = pack_gates_packed(weights)
    aux = np.zeros((128, 2052), np.float32)
    aux[:, 0:2048] = cz_mask_X()
    aux[:, 2048:2052] = sign_vec()
    maps = []
    for i in range(NCORES):
        maps.append({"lr": build_lr30(x[i * BL:(i + 1) * BL]),
                     "gates": gates, "aux": aux})
    return maps


def kernel(x, weights):
    from concourse.bass_utils import run_bass_kernel_spmd

    x = np.ascontiguousarray(np.asarray(x, np.float32))
    weights = np.asarray(weights, np.float32)
    if "nc" not in _CACHE:
        _CACHE["nc"] = _build_nc()
    nc = _CACHE["nc"]
    in_maps = _make_in_maps(x, weights)
    res = run_bass_kernel_spmd(nc, in_maps, core_ids=list(range(NCORES)))
    # y[beta, jj*2+c] -> batch b = c*16 + jj*2 + beta
    b = np.arange(BL)
    sel = (b & 1, ((b >> 1) & 7) * 2 + (b >> 4))
    out = np.concatenate([res.results[i]["y"][sel] for i in range(NCORES)])
    return out.astype(np.float32)


def run_traced(x, weights):
    """Run with NTFF tracing enabled; returns BassKernelResults (for test.py)."""
    from concourse.bass_utils import run_bass_kernel_spmd

    x = np.ascontiguousarray(np.asarray(x, np.float32))
    weights = np.asarray(weights, np.float32)
    if "nc" not in _CACHE:
        _CACHE["nc"] = _build_nc()
    return run_bass_kernel_spmd(_CACHE["nc"], _make_in_maps(x, weights),
                                core_ids=list(range(NCORES)), trace=True)
